# revision 1
# baseline (speedup 1.0000x reference)
"""Trainium2 Bass/Tile kernel for a pre-norm causal decoder block.

Math (matches the jax reference):
    h   = LN1(x) * g1 + beta1
    q,k,v = per-head projections of h (D_HEAD=21, 6 heads)
    sT  = (k @ q^T) / sqrt(21) + causal mask        (scores, transposed)
    e   = exp(sT)                                   (no max-subtraction; scores are tiny)
    o   = (e^T @ [v | 1]) -> per-(t,head) denominator in the appended column
    att = (o / denom) @ Wo + bo
    x1  = x + att
    out = x1 + relu(LN2(x1)*g2+beta2 @ W1 + b1) @ W2 + b2

Sharding: pure data parallelism, batch 512 -> 64 per core across 8 cores.

Layout strategy (per core):
  - tokens T=128 occupy SBUF partitions for LN/residual phases
  - hh is transposed on the PE so q/k/v projections contract over d
  - qT/kT are stored head-padded to 32 partitions (4 heads in "A" [128,*],
    2 heads in "B" [64,*]) so score matmuls are K=32 row-tiles
  - scores are computed transposed (sT[s,t]) so the softmax denominator is
    a matmul-accumulated ones-column and no attention transpose is needed
  - causal mask is added in-PSUM via an identity matmul (values -30 => exp ~ 1e-13)
  - all matmul operands bf16, PSUM accumulation fp32, LN/softmax arithmetic fp32
"""

import os
import numpy as np
import ml_dtypes

from contextlib import ExitStack

import concourse.bass as bass
import concourse.bacc as bacc
import concourse.tile as tile
from concourse import mybir
from concourse.bass_utils import run_bass_kernel_spmd

BF = mybir.dt.bfloat16
F32 = mybir.dt.float32
NPBF = ml_dtypes.bfloat16

B, T, D = 512, 128, 128
NH, DH = 6, 21
DC = NH * DH  # 126
DFF = 512
NCORES = 8
BPC = B // NCORES  # 64 batches per core
G = 4              # batches per group (free-dim batching of qkv projections)
EPS = 1e-5
MASK_NEG = -30.0
SM_SCALE = 1.0 / np.sqrt(np.float32(DH))

AF = mybir.ActivationFunctionType
ALU = mybir.AluOpType
AX = mybir.AxisListType


def _bf(a):
    return np.ascontiguousarray(np.asarray(a, dtype=np.float32)).astype(NPBF)


def _prep_weights(Wq, Wk, Wv, Wo, bo, W1, b1, W2, b2, g1, beta1, g2, beta2):
    """Host-side folding/packing. Returns dict of named arrays + flags."""
    Wq = np.asarray(Wq, np.float64)
    Wk = np.asarray(Wk, np.float64)
    Wv = np.asarray(Wv, np.float64)
    g1 = np.asarray(g1, np.float64)
    g2 = np.asarray(g2, np.float64)
    beta1 = np.asarray(beta1, np.float64)
    beta2 = np.asarray(beta2, np.float64)
    W1 = np.asarray(W1, np.float64)

    # fold g1 into the qkv projections, 1/sqrt(DH) into Wq
    Wq_f = g1[None, :, None] * Wq * SM_SCALE   # [h, d, e]
    Wk_f = g1[None, :, None] * Wk
    Wv_f = g1[None, :, None] * Wv

    # pair-packed q/k: tensor i holds heads 2i (rows 0..20) and 2i+1 (rows 32..52)
    # 64-partition tensors limit PE row-tile concurrency to 2, and adjacent
    # heads' score matmuls alternate PSUM banks (same-bank concurrent PE
    # writes crash the device)
    wq_p = np.zeros((3, D, 64), np.float64)
    wk_p = np.zeros((3, D, 64), np.float64)
    for h in range(NH):
        i, off = h // 2, 32 * (h % 2)
        wq_p[i, :, off:off + DH] = Wq_f[h]
        wk_p[i, :, off:off + DH] = Wk_f[h]
    wv = np.concatenate([Wv_f[h] for h in range(NH)], axis=1)  # [128, 126]

    # beta1 contributions (rank-1 into qT/kT/v)
    qb = np.einsum("d,hde->he", beta1, Wq) * SM_SCALE   # [6, 21]
    kb = np.einsum("d,hde->he", beta1, Wk)
    vb = np.einsum("d,hde->he", beta1, Wv)
    qb_p = np.zeros((64, 3), np.float64)
    kb_p = np.zeros((64, 3), np.float64)
    for h in range(NH):
        i, off = h // 2, 32 * (h % 2)
        qb_p[off:off + DH, i] = qb[h]
        kb_p[off:off + DH, i] = kb[h]
    vb_r = vb.reshape(1, DC)

    w1 = g2[:, None] * W1                     # [128, 512]
    b1_eff = np.asarray(b1, np.float64) + beta2 @ W1   # [512]
    w2 = np.asarray(W2, np.float64).reshape(4, 128, D).transpose(1, 0, 2)  # [128,4,128]

    # additive causal mask (pre-exp): 0 where s <= t else -30
    mask1 = np.where(np.arange(T)[:, None] <= np.arange(T)[None, :], 0.0, MASK_NEG)
    mask3 = np.tile(mask1, (1, 3))            # [T, 384] for one score bank

    out = {
        "wq_p": _bf(wq_p), "wk_p": _bf(wk_p),
        "wv": _bf(wv),
        "wo": _bf(Wo), "w1": _bf(w1), "w2": _bf(w2),
        "mask3": _bf(mask3),
        "ident": _bf(np.eye(128)),
        "qb_p": np.asarray(qb_p, np.float32),
        "kb_p": np.asarray(kb_p, np.float32),
        "vb_r": np.asarray(vb_r, np.float32),
        "bo_r": np.asarray(bo, np.float32).reshape(1, D),
        "b2_r": np.asarray(b2, np.float32).reshape(1, D),
        "b1e": np.ascontiguousarray(
            np.asarray(b1_eff, np.float64).reshape(4, 128).T, dtype=np.float32
        ),  # [128, 4] per-partition relu bias per chunk
    }
    flags = {
        "qkv_bias": bool(np.any(beta1 != 0.0)),
        "bo": bool(np.any(np.asarray(bo) != 0.0)),
        "b2": bool(np.any(np.asarray(b2) != 0.0)),
        "b1": bool(np.any(out["b1e"] != 0.0)),
    }
    return out, flags


def _emit(ctx, tc, aps, flags, bpc):
    nc = tc.nc
    x_ap = aps["x"]
    y_ap = aps["y"]

    singles = ctx.enter_context(tc.tile_pool(name="singles", bufs=1))
    sb_g = ctx.enter_context(tc.tile_pool(name="sb_g", bufs=4))
    sb_b = ctx.enter_context(tc.tile_pool(name="sb_b", bufs=6))
    sb_s = ctx.enter_context(tc.tile_pool(name="sb_s", bufs=12))
    psg = ctx.enter_context(tc.tile_pool(name="psg", bufs=2, space="PSUM"))
    pss = ctx.enter_context(tc.tile_pool(name="pss", bufs=3, space="PSUM"))
    psb = ctx.enter_context(tc.tile_pool(name="psb", bufs=2, space="PSUM"))
    psf = ctx.enter_context(tc.tile_pool(name="psf", bufs=1, space="PSUM"))

    # ---- resident constants -------------------------------------------------
    def load_const(name, shape, dtype=BF):
        t = singles.tile(list(shape), dtype, tag=name)
        nc.sync.dma_start(out=t[:], in_=aps[name])
        return t

    wq_p = singles.tile([D, 3, 64], BF, tag="wq_p")
    nc.sync.dma_start(out=wq_p[:], in_=aps["wq_p"].rearrange("i d e -> d i e"))
    wk_p = singles.tile([D, 3, 64], BF, tag="wk_p")
    nc.sync.dma_start(out=wk_p[:], in_=aps["wk_p"].rearrange("i d e -> d i e"))
    wv = load_const("wv", [D, DC])
    wo = load_const("wo", [DC, D])
    w1 = load_const("w1", [D, DFF])
    w2 = load_const("w2", [D, 4, D])
    mask3 = load_const("mask3", [T, 384])
    ident = load_const("ident", [128, 128])
    if flags["qkv_bias"]:
        qb_p = load_const("qb_p", [64, 3], F32)
        kb_p = load_const("kb_p", [64, 3], F32)
        vb_rep = singles.tile([128, DC], F32, tag="vb_rep")
        nc.sync.dma_start(out=vb_rep[:], in_=aps["vb_r"].to_broadcast([128, DC]))
    if flags["bo"]:
        bo_rep = singles.tile([128, D], F32, tag="bo_rep")
        nc.sync.dma_start(out=bo_rep[:], in_=aps["bo_r"].to_broadcast([128, D]))
    if flags["b2"]:
        b2_rep = singles.tile([128, D], F32, tag="b2_rep")
        nc.sync.dma_start(out=b2_rep[:], in_=aps["b2_r"].to_broadcast([128, D]))
    if flags["b1"]:
        b1e = load_const("b1e", [128, 4], F32)

    eps_t = singles.tile([128, 1], F32, tag="eps")
    nc.vector.memset(eps_t[:], EPS)

    n_groups = bpc // G
    repeat = int(os.environ.get("K_REPEAT", "1"))
    glist = [gg for _ in range(repeat) for gg in range(n_groups)]

    def load_x(g):
        x_t = sb_g.tile([T, G, D], F32, tag="x_t")
        nc.sync.dma_start(
            out=x_t[:], in_=x_ap[g * G:(g + 1) * G].rearrange("b t d -> t b d")
        )
        return x_t

    def phase_A(x_t):
        """LN1 + transpose + q/k/v projections for one group."""
        st1 = sb_s.tile([128, G, 6], F32, tag="st1")
        for b in range(G):
            nc.vector.bn_stats(out=st1[:, b, :], in_=x_t[:, b, :])
        mean1 = sb_s.tile([128, G], F32, tag="mean1")
        var1 = sb_s.tile([128, G], F32, tag="var1")
        rstd1 = sb_s.tile([128, G, 1], F32, tag="rstd1")
        dd = sb_s.tile([128, G], F32, tag="dd")
        # mean = (m_e + m_o)/2 ; var = (M2_e + M2_o)/D + (m_e - m_o)^2/4
        nc.vector.tensor_add(out=mean1[:], in0=st1[:, :, 1], in1=st1[:, :, 4])
        nc.vector.tensor_scalar_mul(out=mean1[:], in0=mean1[:], scalar1=0.5)
        nc.vector.tensor_sub(out=dd[:], in0=st1[:, :, 1], in1=st1[:, :, 4])
        nc.vector.tensor_mul(out=dd[:], in0=dd[:], in1=dd[:])
        nc.vector.tensor_add(out=var1[:], in0=st1[:, :, 2], in1=st1[:, :, 5])
        nc.vector.tensor_scalar(
            out=var1[:], in0=var1[:], scalar1=1.0 / D, scalar2=None, op0=ALU.mult,
        )
        nc.vector.tensor_scalar(
            out=dd[:], in0=dd[:], scalar1=0.25, scalar2=None, op0=ALU.mult,
        )
        nc.vector.tensor_add(out=var1[:], in0=var1[:], in1=dd[:])
        nc.scalar.activation(
            out=rstd1[:], in_=var1[:].rearrange("p (g o) -> p g o", o=1),
            func=AF.Sqrt, bias=eps_t[:], scale=1.0,
        )
        nc.vector.reciprocal(out=rstd1[:], in_=rstd1[:])

        hh = sb_g.tile([T, G, D], BF, tag="hh")
        for b in range(G):
            nc.gpsimd.tensor_scalar(
                out=hh[:, b, :], in0=x_t[:, b, :],
                scalar1=mean1[:, b:b + 1], scalar2=rstd1[:, b, :],
                op0=ALU.subtract, op1=ALU.mult,
            )

        hhT = sb_g.tile([D, G, T], BF, tag="hhT")
        for b in range(G):
            tp = psg.tile([128, 128], BF, tag="gp")
            nc.tensor.transpose(out=tp[:], in_=hh[:, b, :], identity=ident[:])
            nc.vector.tensor_copy(out=hhT[:, b, :], in_=tp[:])

        def proj(w_slice):
            ps = psg.tile([64, G * T], F32, tag="gp")
            nc.tensor.matmul(
                ps[:], w_slice, hhT[:].rearrange("d b t -> d (b t)"),
                start=True, stop=True,
            )
            return ps

        qt = sb_g.tile([64, 3, G, T], BF, tag="qt")
        kt = sb_g.tile([64, 3, G, T], BF, tag="kt")
        for i in range(3):
            q_ps = proj(wq_p[:, i, :])
            if flags["qkv_bias"]:
                nc.scalar.activation(
                    out=qt[:, i, :, :].rearrange("p b t -> p (b t)"), in_=q_ps[:],
                    func=AF.Identity, bias=qb_p[:, i:i + 1], scale=1.0,
                )
            else:
                nc.scalar.copy(
                    out=qt[:, i, :, :].rearrange("p b t -> p (b t)"), in_=q_ps[:]
                )
            k_ps = proj(wk_p[:, i, :])
            if flags["qkv_bias"]:
                nc.vector.tensor_scalar_add(
                    out=kt[:, i, :, :].rearrange("p b t -> p (b t)"), in0=k_ps[:],
                    scalar1=kb_p[:, i:i + 1],
                )
            else:
                nc.vector.tensor_copy(
                    out=kt[:, i, :, :].rearrange("p b t -> p (b t)"), in_=k_ps[:]
                )

        v_ps = psg.tile([T, G, NH, DH], F32, tag="gp")
        for b in range(G):
            nc.tensor.matmul(
                v_ps[:, b, :, :], hhT[:, b, :], wv[:],
                start=True, stop=True, skip_group_check=True,
            )
        v_sb = sb_g.tile([T, G, NH, DH + 1], BF, tag="v_sb")
        if flags["qkv_bias"]:
            vb3 = vb_rep[:].rearrange("p (h e) -> p h e", h=NH)
            vb4 = bass.AP(
                tensor=vb3.tensor, offset=vb3.offset,
                ap=[vb3.ap[0], [0, G], vb3.ap[1], vb3.ap[2]],
            )
            nc.vector.tensor_tensor(
                out=v_sb[:, :, :, 0:DH], in0=v_ps[:], in1=vb4, op=ALU.add,
            )
        else:
            nc.vector.tensor_copy(out=v_sb[:, :, :, 0:DH], in_=v_ps[:])
        nc.gpsimd.memset(v_sb[:, :, :, DH:DH + 1], 1.0)
        return qt, kt, v_sb

    def phase_B(x_t, qt, kt, v_sb):
        """Causal attention + residual for the G batches of one group."""
        x1_all = sb_b.tile([T, G, D], F32, tag="x1")
        for b in range(G):
            sA = pss.tile([T, 384], F32, tag="sT")
            sB = pss.tile([T, 384], F32, tag="sT")
            banks = (sA, sB)
            # mask lands first (start=True over the whole bank); score matmuls
            # then accumulate onto it, one per 128-col region
            for s_ps in (sA, sB):
                nc.tensor.matmul(
                    s_ps[:], ident[:], mask3[:],
                    start=True, stop=False, skip_group_check=True,
                )
            for h in range(NH):
                i, off = h // 2, 32 * (h % 2)
                nc.tensor.matmul(
                    banks[h % 2][:, 128 * i:128 * (i + 1)],
                    kt[off:off + 32, i, b, :],
                    qt[off:off + 32, i, b, :],
                    start=False, stop=(h >= 4), skip_group_check=True,
                )
            eT = sb_b.tile([T, NH * T], BF, tag="eT")
            nc.scalar.activation(out=eT[:, 0:384], in_=sA[:], func=AF.Exp)
            nc.scalar.activation(out=eT[:, 384:768], in_=sB[:], func=AF.Exp)

            # o[t, (h, e+1)] with the softmax denominator in the last column
            o_ps = psb.tile([T, NH, DH + 1], F32, tag="bp")
            for h in range(NH):
                ecol = 384 * (h % 2) + 128 * (h // 2)
                nc.tensor.matmul(
                    o_ps[:, h, :],
                    eT[:, ecol:ecol + 128],
                    v_sb[:, b, h, :],
                    start=True, stop=True, skip_group_check=True,
                )

            recip = sb_s.tile([128, NH, 1], F32, tag="recip")
            nc.vector.reciprocal(out=recip[:], in_=o_ps[:, :, DH:DH + 1])
            o_sb = sb_b.tile([T, NH, DH], BF, tag="o_sb")
            nc.vector.tensor_tensor(
                out=o_sb[:], in0=o_ps[:, :, 0:DH],
                in1=recip[:].to_broadcast([128, NH, DH]), op=ALU.mult,
            )

            oT_ps = psb.tile([DC, T], BF, tag="bp")
            nc.tensor.transpose(
                out=oT_ps[:], in_=o_sb[:].rearrange("t h e -> t (h e)"),
                identity=ident[:],
            )
            oT_sb = sb_b.tile([DC, T], BF, tag="oT")
            nc.vector.tensor_copy(out=oT_sb[:], in_=oT_ps[:])

            att_ps = psb.tile([T, D], F32, tag="bp")
            nc.tensor.matmul(att_ps[:], oT_sb[:], wo[:], start=True, stop=True)

            nc.vector.tensor_add(out=x1_all[:, b, :], in0=x_t[:, b, :], in1=att_ps[:])
            if flags["bo"]:
                nc.vector.tensor_add(
                    out=x1_all[:, b, :], in0=x1_all[:, b, :], in1=bo_rep[:]
                )
        return x1_all

    def phase_C(g, x1_all):
        """LN2 + feed-forward + residual + store for one group."""
        st2 = sb_s.tile([128, G, 6], F32, tag="st2")
        for b in range(G):
            nc.vector.bn_stats(out=st2[:, b, :], in_=x1_all[:, b, :])
        mean2 = sb_s.tile([128, G], F32, tag="mean2")
        var2 = sb_s.tile([128, G], F32, tag="var2")
        rstd2 = sb_s.tile([128, G, 1], F32, tag="rstd2")
        dd2 = sb_s.tile([128, G], F32, tag="dd2")
        nc.vector.tensor_add(out=mean2[:], in0=st2[:, :, 1], in1=st2[:, :, 4])
        nc.vector.tensor_scalar_mul(out=mean2[:], in0=mean2[:], scalar1=0.5)
        nc.vector.tensor_sub(out=dd2[:], in0=st2[:, :, 1], in1=st2[:, :, 4])
        nc.vector.tensor_mul(out=dd2[:], in0=dd2[:], in1=dd2[:])
        nc.vector.tensor_add(out=var2[:], in0=st2[:, :, 2], in1=st2[:, :, 5])
        nc.vector.tensor_scalar(
            out=var2[:], in0=var2[:], scalar1=1.0 / D, scalar2=None, op0=ALU.mult,
        )
        nc.vector.tensor_scalar(
            out=dd2[:], in0=dd2[:], scalar1=0.25, scalar2=None, op0=ALU.mult,
        )
        nc.vector.tensor_add(out=var2[:], in0=var2[:], in1=dd2[:])
        nc.scalar.activation(
            out=rstd2[:], in_=var2[:].rearrange("p (g o) -> p g o", o=1),
            func=AF.Sqrt, bias=eps_t[:], scale=1.0,
        )
        nc.vector.reciprocal(out=rstd2[:], in_=rstd2[:])

        for b in range(G):
            hh2 = sb_b.tile([T, D], BF, tag="hh2")
            nc.gpsimd.tensor_scalar(
                out=hh2[:], in0=x1_all[:, b, :],
                scalar1=mean2[:, b:b + 1], scalar2=rstd2[:, b, :],
                op0=ALU.subtract, op1=ALU.mult,
            )
            h2T_ps = psf.tile([D, T], BF, tag="fp")
            nc.tensor.transpose(out=h2T_ps[:], in_=hh2[:], identity=ident[:])
            h2T = sb_b.tile([D, T], BF, tag="h2T")
            nc.vector.tensor_copy(out=h2T[:], in_=h2T_ps[:])

            ff1_ps = psf.tile([128, 4, T], F32, tag="fp")
            for c in range(4):
                nc.tensor.matmul(
                    ff1_ps[:, c, :], w1[:, 128 * c:128 * (c + 1)], h2T[:],
                    start=True, stop=True, skip_group_check=True,
                )
            r_sb = sb_b.tile([128, 4, T], BF, tag="r_sb")
            if flags["b1"]:
                for c in range(4):
                    nc.scalar.activation(
                        out=r_sb[:, c, :], in_=ff1_ps[:, c, :], func=AF.Relu,
                        bias=b1e[:, c:c + 1], scale=1.0,
                    )
            else:
                nc.scalar.activation(
                    out=r_sb[:].rearrange("p c t -> p (c t)"),
                    in_=ff1_ps[:].rearrange("p c t -> p (c t)"),
                    func=AF.Relu,
                )

            ff2_ps = psf.tile([T, D], F32, tag="fp")
            for c in range(4):
                nc.tensor.matmul(
                    ff2_ps[:], r_sb[:, c, :], w2[:, c, :],
                    start=(c == 0), stop=(c == 3),
                )

            out_sb = sb_b.tile([T, D], F32, tag="out_sb")
            nc.vector.tensor_add(out=out_sb[:], in0=x1_all[:, b, :], in1=ff2_ps[:])
            if flags["b2"]:
                nc.vector.tensor_add(out=out_sb[:], in0=out_sb[:], in1=b2_rep[:])
            nc.gpsimd.dma_start(out=y_ap[g * G + b], in_=out_sb[:])

    # software pipeline: B(g) | A(g+1) | C(g)
    x_cur = load_x(glist[0])
    A_cur = phase_A(x_cur)
    for gi, g in enumerate(glist):
        more = gi + 1 < len(glist)
        if more:
            x_nxt = load_x(glist[gi + 1])
        x1 = phase_B(x_cur, *A_cur)
        if more:
            A_nxt = phase_A(x_nxt)
        phase_C(g, x1)
        if more:
            x_cur, A_cur = x_nxt, A_nxt


def build_program(weights, flags, bpc=BPC):
    nc = bacc.Bacc("TRN2", target_bir_lowering=False, debug=False)
    aps = {}
    aps["x"] = nc.dram_tensor("x", [bpc, T, D], F32, kind="ExternalInput").ap()
    aps["y"] = nc.dram_tensor("y", [bpc, T, D], F32, kind="ExternalOutput").ap()
    for name, arr in weights.items():
        dt = F32 if arr.dtype == np.float32 else BF
        aps[name] = nc.dram_tensor(name, list(arr.shape), dt, kind="ExternalInput").ap()
    with tile.TileContext(nc) as tc:
        with ExitStack() as ctx:
            _emit(ctx, tc, aps, flags, bpc)
    nc.compile()
    return nc


_CACHE = {}


def _get_program_and_maps(x, args):
    x = np.asarray(x, np.float32)
    weights, flags = _prep_weights(*args)
    key = tuple(sorted(flags.items()))
    if key not in _CACHE:
        _CACHE[key] = build_program(weights, flags)
    nc = _CACHE[key]
    in_maps = []
    for c in range(NCORES):
        m = {"x": np.ascontiguousarray(x[c * BPC:(c + 1) * BPC])}
        m.update(weights)
        in_maps.append(m)
    return nc, in_maps


def kernel(x, Wq, Wk, Wv, Wo, bo, W1, b1, W2, b2, g1, beta1, g2, beta2):
    nc, in_maps = _get_program_and_maps(
        x, (Wq, Wk, Wv, Wo, bo, W1, b1, W2, b2, g1, beta1, g2, beta2)
    )
    res = run_bass_kernel_spmd(nc, in_maps, list(range(NCORES)))
    out = np.concatenate([res.results[c]["y"] for c in range(NCORES)], axis=0)
    return out.astype(np.float32)


def run_traced(inputs):
    """Profiled run; returns BassKernelResults with exec_time_ns if available."""
    args = tuple(
        inputs[k]
        for k in ("Wq", "Wk", "Wv", "Wo", "bo", "W1", "b1", "W2", "b2",
                  "g1", "beta1", "g2", "beta2")
    )
    nc, in_maps = _get_program_and_maps(inputs["x"], args)
    return run_bass_kernel_spmd(nc, in_maps, list(range(NCORES)), trace=True)



# revision 19
# speedup vs baseline: 1.3317x; 1.3317x over previous
"""Trainium2 Bass/Tile kernel for a pre-norm causal decoder block.

Math (matches the jax reference):
    h   = LN1(x) * g1 + beta1
    q,k,v = per-head projections of h (D_HEAD=21, 6 heads)
    sT  = (k @ q^T) / sqrt(21)                       (scores, transposed)
    e   = exp(sT) * tril01                           (multiplicative causal mask)
    o   = (e^T @ [v | 1]) -> softmax denominator in the appended column
    x1  = x + (o / denom) @ Wo + bo
    out = x1 + relu(LN2(x1) @ (g2*W1) + b1eff) @ W2 + b2

Sharding: pure data parallelism, batch 512 -> 64 per core across 8 cores.

Layout strategy (per core) - the residual stream lives TRANSPOSED as
xT[d, (b, t)] so model-dim contractions never need PE transposes:
  - x is loaded with a casting transposing DMA (f32 -> bf16, "b t d -> d b t")
  - LN statistics are ones-vector matmuls over the partition (d) axis;
    mean/var/rstd are computed once for all 64 batches on [1, 8192] rows
    (rstd = exp(-0.5*ln(var+eps)) keeps the Act engine on a single
    activation-table set: natural_log_exp covers Ln/Exp/Relu)
  - scores use K-stacked stationaries (4 heads x 32 rows = 128) with a
    block-diagonal zero-padded moving operand, so one matmul emits four
    heads' scores; heads 4-5 go in a second K=64 matmul
  - the causal mask is a multiplicative 0/1 tril applied post-exp on DVE
    (no mask matmuls, no -inf arithmetic)
  - attention output o[t,(h,e+1)] carries the softmax denominator in an
    appended ones column of v; Wo and the FF matmuls are group-batched
    (moving N=512) and both residual adds ride as identity-matmul
    preloads of the PSUM accumulators
All matmul operands bf16, PSUM accumulation fp32.
"""

import os
import numpy as np
import ml_dtypes

from contextlib import ExitStack

import concourse.bass as bass
import concourse.bacc as bacc
import concourse.tile as tile
from concourse import mybir
from concourse.bass_utils import run_bass_kernel_spmd

BF = mybir.dt.bfloat16
F32 = mybir.dt.float32
NPBF = ml_dtypes.bfloat16

B, T, D = 512, 128, 128
NH, DH = 6, 21
DC = NH * DH  # 126
DFF = 512
NCORES = 8
BPC = B // NCORES  # 64 batches per core
G = 4              # batches per group
NG = BPC // G      # 16 groups
EPS = 1e-5
SM_SCALE = 1.0 / np.sqrt(np.float32(DH))

AF = mybir.ActivationFunctionType
ALU = mybir.AluOpType


def _bf(a):
    return np.ascontiguousarray(np.asarray(a, dtype=np.float32)).astype(NPBF)


def _prep_weights(Wq, Wk, Wv, Wo, bo, W1, b1, W2, b2, g1, beta1, g2, beta2):
    """Host-side folding/packing. Returns dict of named arrays + flags."""
    Wq = np.asarray(Wq, np.float64)
    Wk = np.asarray(Wk, np.float64)
    Wv = np.asarray(Wv, np.float64)
    g1 = np.asarray(g1, np.float64)
    g2 = np.asarray(g2, np.float64)
    beta1 = np.asarray(beta1, np.float64)
    beta2 = np.asarray(beta2, np.float64)
    W1 = np.asarray(W1, np.float64)

    Wq_f = g1[None, :, None] * Wq * SM_SCALE   # [h, d, e]
    Wk_f = g1[None, :, None] * Wk
    Wv_f = g1[None, :, None] * Wv

    # K-stacked projection weights: 4 heads (rows 32h..32h+20) and 2 heads
    wq4 = np.zeros((D, 128), np.float64)
    wk4 = np.zeros((D, 128), np.float64)
    wq2 = np.zeros((D, 64), np.float64)
    wk2 = np.zeros((D, 64), np.float64)
    for h in range(4):
        wq4[:, 32 * h:32 * h + DH] = Wq_f[h]
        wk4[:, 32 * h:32 * h + DH] = Wk_f[h]
    for h in range(2):
        wq2[:, 32 * h:32 * h + DH] = Wq_f[4 + h]
        wk2[:, 32 * h:32 * h + DH] = Wk_f[4 + h]
    wv = np.concatenate([Wv_f[h] for h in range(NH)], axis=1)  # [128, 126]

    # beta1 contributions (per-stack-row biases for q/k; per-(h,e) row for v)
    qb = np.einsum("d,hde->he", beta1, Wq) * SM_SCALE   # [6, 21]
    kb = np.einsum("d,hde->he", beta1, Wk)
    vb = np.einsum("d,hde->he", beta1, Wv)
    qb4 = np.zeros((128, 1), np.float64)
    kb4 = np.zeros((128, 1), np.float64)
    qb2 = np.zeros((64, 1), np.float64)
    kb2 = np.zeros((64, 1), np.float64)
    for h in range(4):
        qb4[32 * h:32 * h + DH, 0] = qb[h]
        kb4[32 * h:32 * h + DH, 0] = kb[h]
    for h in range(2):
        qb2[32 * h:32 * h + DH, 0] = qb[4 + h]
        kb2[32 * h:32 * h + DH, 0] = kb[4 + h]

    w1 = g2[:, None] * W1                     # [128, 512]
    b1_eff = np.asarray(b1, np.float64) + beta2 @ W1   # [512]
    w2c = np.asarray(W2, np.float64).reshape(4, 128, D).transpose(1, 0, 2)  # [128,4,128]

    mask01 = np.where(
        np.arange(T)[:, None] <= np.arange(T)[None, :], 1.0, 0.0
    )  # [s, t] keep where s <= t

    out = {
        "wq4": _bf(wq4), "wq2": _bf(wq2), "wk4": _bf(wk4), "wk2": _bf(wk2),
        "wv": _bf(wv), "wo": _bf(Wo), "w1": _bf(w1), "w2c": _bf(w2c),
        "mask01": _bf(mask01), "ident": _bf(np.eye(128)),
        "ones_bf": _bf(np.ones((D, 1))),
        "qb4": np.asarray(qb4, np.float32), "qb2": np.asarray(qb2, np.float32),
        "kb4": np.asarray(kb4, np.float32), "kb2": np.asarray(kb2, np.float32),
        "vb_r": np.asarray(vb.reshape(1, DC), np.float32),
        "bo_c": np.asarray(bo, np.float32).reshape(D, 1),
        "b2_c": np.asarray(b2, np.float32).reshape(D, 1),
        "beta1_c": np.asarray(beta1, np.float32).reshape(D, 1),
        "b1e": np.ascontiguousarray(
            np.asarray(b1_eff, np.float64).reshape(4, 128).T, dtype=np.float32
        ),  # [128, 4] per-partition relu bias per chunk
    }
    flags = {
        "qkv_bias": bool(np.any(beta1 != 0.0)),
        "beta1": bool(np.any(beta1 != 0.0)),
        "bo": bool(np.any(np.asarray(bo) != 0.0)),
        "b2": bool(np.any(np.asarray(b2) != 0.0)),
        "b1": bool(np.any(out["b1e"] != 0.0)),
    }
    return out, flags


def _emit(ctx, tc, aps, flags, bpc):
    nc = tc.nc
    x_ap = aps["x"]
    y_ap = aps["y"]

    singles = ctx.enter_context(tc.tile_pool(name="singles", bufs=1))
    sbg = ctx.enter_context(tc.tile_pool(name="sbg", bufs=2))
    psA = ctx.enter_context(tc.tile_pool(name="psA", bufs=1, space="PSUM"))
    psS = ctx.enter_context(tc.tile_pool(name="psS", bufs=1, space="PSUM"))
    psO = ctx.enter_context(tc.tile_pool(name="psO", bufs=1, space="PSUM"))
    psM = ctx.enter_context(tc.tile_pool(name="psM", bufs=1, space="PSUM"))
    psF = ctx.enter_context(tc.tile_pool(name="psF", bufs=1, space="PSUM"))

    def load_const(name, shape, dtype=BF):
        t = singles.tile(list(shape), dtype, tag=name, name=name)
        nc.sync.dma_start(out=t[:], in_=aps[name])
        return t

    wq4 = load_const("wq4", [D, 128])
    wq2 = load_const("wq2", [D, 64])
    wk4 = load_const("wk4", [D, 128])
    wk2 = load_const("wk2", [D, 64])
    wv = load_const("wv", [D, DC])
    wo = load_const("wo", [DC, D])
    w1 = load_const("w1", [D, DFF])
    w2c = load_const("w2c", [D, 4, D])
    mask01 = load_const("mask01", [T, T])
    ident = load_const("ident", [128, 128])
    ones_bf = load_const("ones_bf", [D, 1])
    if flags["qkv_bias"]:
        qb4 = load_const("qb4", [128, 1], F32)
        qb2 = load_const("qb2", [64, 1], F32)
        kb4 = load_const("kb4", [128, 1], F32)
        kb2 = load_const("kb2", [64, 1], F32)
        vb_r = load_const("vb_r", [1, DC], F32)
    if flags["beta1"]:
        beta1_c = load_const("beta1_c", [D, 1], F32)
    if flags["bo"]:
        bo_c = load_const("bo_c", [D, 1], F32)
    if flags["b2"]:
        b2_c = load_const("b2_c", [D, 1], F32)
    if flags["b1"]:
        b1e = load_const("b1e", [128, 4], F32)

    # ---- whole-core resident tensors -------------------------------------
    stage = singles.tile([T, bpc, D], BF, tag="stage")   # load/store staging
    xb, oS = stage, stage
    xTb = singles.tile([D, bpc, T], BF, tag="xTb")       # bf16 residual in
    xx1b = singles.tile([D, bpc, T], BF, tag="xx1b")     # post-attn residual
    outT = singles.tile([D, bpc, T], BF, tag="outT")     # final output (T)
    # LN stat rows (all on partition 0; shared by LN1/LN2 - the per-group
    # broadcasts read the DRAM copies)
    st_sums = singles.tile([1, NG * 512], BF, tag="st_sums")
    st_sq = singles.tile([1, NG * 512], BF, tag="st_sq")
    st_mu = singles.tile([1, NG * 512], BF, tag="st_mu")
    st_rstd = singles.tile([1, NG * 512], BF, tag="st_rstd")

    # block-diagonal moving operands for the K-stacked score matmuls;
    # off-block zeros are written once and never touched again
    qblk4 = singles.tile([128, G, 4, T], BF, tag="qblk4")
    qblk2 = singles.tile([64, G, 2, T], BF, tag="qblk2")
    v_sb = singles.tile([T, G, NH, DH + 1], BF, tag="v_sb")
    k4sb = singles.tile([128, G, T], BF, tag="k4sb")
    k2sb = singles.tile([64, G, T], BF, tag="k2sb")

    nc.vector.memset(qblk4[:], 0.0)
    nc.vector.memset(qblk2[:], 0.0)
    nc.gpsimd.memset(v_sb[:, :, :, DH:DH + 1], 1.0)
    eps_t = singles.tile([4, 1], F32, tag="eps")
    nc.vector.memset(eps_t[:], EPS)

    NQ = 4           # DMA quarters
    BQ = bpc // NQ   # 16 batches per quarter

    # eT column offset of head h for pair-slot j (see spair bank layout)
    def ecol(j, h):
        if j == 0:
            return 128 * h if h < 4 else 512 + 128 * (h - 4)
        return 1024 + 128 * h if h < 4 else 768 + 128 * (h - 4)

    stats1_dram = nc.dram_tensor("stats1_dram", [2, NG * 512], BF, kind="Internal").ap()
    stats2_dram = nc.dram_tensor("stats2_dram", [2, NG * 512], BF, kind="Internal").ap()

    def ln_stats(src, stats_dram):
        """Per-(b,t)-column mean/rstd over the partition (d) axis."""
        for g in range(NG):
            cols = slice(512 * g, 512 * (g + 1))
            xsqg = sbg.tile([D, G, T], BF, tag="xsqg", name="xsqg")
            s = src[:, G * g:G * (g + 1), :]
            nc.vector.tensor_tensor(out=xsqg[:], in0=s, in1=s, op=ALU.mult)
            stp = psM.tile([1, 512], F32, tag="m", name="stp")
            nc.tensor.matmul(
                stp[:], ones_bf[:],
                s.rearrange("d g t -> d (g t)"),
                start=True, stop=True, skip_group_check=True,
            )
            stp2 = psO.tile([1, 512], F32, tag="o", name="stp2")
            nc.tensor.matmul(
                stp2[:], ones_bf[:],
                xsqg[:].rearrange("d g t -> d (g t)"),
                start=True, stop=True, skip_group_check=True,
            )
            nc.vector.tensor_copy(out=st_sums[0:1, cols], in_=stp[:])
            nc.vector.tensor_copy(out=st_sq[0:1, cols], in_=stp2[:])
        nc.vector.tensor_scalar_mul(out=st_mu[:], in0=st_sums[:], scalar1=1.0 / D)
        nc.vector.tensor_scalar_mul(out=st_sq[:], in0=st_sq[:], scalar1=1.0 / D)
        nc.vector.tensor_mul(out=st_sums[:], in0=st_mu[:], in1=st_mu[:])
        nc.vector.tensor_sub(out=st_sq[:], in0=st_sq[:], in1=st_sums[:])
        nc.scalar.activation(out=st_sq[:], in_=st_sq[:], func=AF.Ln, bias=eps_t[0:1, :])
        nc.scalar.activation(out=st_rstd[:], in_=st_sq[:], func=AF.Exp, scale=-0.5)
        nc.sync.dma_start(out=stats_dram[0:1, :], in_=st_mu[:])
        nc.sync.dma_start(out=stats_dram[1:2, :], in_=st_rstd[:])

    def normalize(src, stats_dram, g, out_t, add_beta1):
        """out_t[d, (g t)] = (src - mu) * rstd  (+ beta1)."""
        cols = slice(512 * g, 512 * (g + 1))
        # DVE cannot broadcast across partitions: replicate the stat rows
        # (via DRAM, whose APs allow a zero partition step)
        rep = sbg.tile([D, 2, 512], BF, tag="rep", name="rep")
        nc.sync.dma_start(
            out=rep[:, 0, :], in_=stats_dram[0:1, cols].to_broadcast([D, 512])
        )
        nc.sync.dma_start(
            out=rep[:, 1, :], in_=stats_dram[1:2, cols].to_broadcast([D, 512])
        )
        s = src[:, G * g:G * (g + 1), :].rearrange("d g t -> d (g t)")
        o = out_t[:].rearrange("d g t -> d (g t)")
        nc.vector.tensor_tensor(out=o, in0=s, in1=rep[:, 0, :], op=ALU.subtract)
        nc.vector.tensor_tensor(out=o, in0=o, in1=rep[:, 1, :], op=ALU.mult)
        if add_beta1:
            nc.vector.tensor_scalar_add(out=o, in0=o, scalar1=beta1_c[:])

    def emit_once():
        # ---- P0: load + LN1 stats ----------------------------------------
        nc.gpsimd.dma_start(out=xb[:], in_=x_ap.rearrange("b t d -> t b d"))
        for b in range(bpc):
            nc.sync.dma_start_transpose(out=xTb[:, b, :], in_=xb[:, b, :])
        ln_stats(xTb, stats1_dram)

        # ---- P1: attention per group -------------------------------------
        for g in range(NG):
            hhT = sbg.tile([D, G, T], BF, tag="hhT", name="hhT")
            normalize(xTb, stats1_dram, g, hhT, flags["beta1"])
            hhflat = hhT[:].rearrange("d g t -> d (g t)")

            # q/k projections (K-stacked rows) + block-diag staging
            q4_ps = psA.tile([128, G, T], F32, tag="a", name="q4_ps")
            nc.tensor.matmul(
                q4_ps[:].rearrange("p g t -> p (g t)"), wq4[:], hhflat,
                start=True, stop=True,
            )
            for h in range(4):
                src = q4_ps[32 * h:32 * h + 32, :, :]
                dst = qblk4[32 * h:32 * h + 32, :, h, :]
                if flags["qkv_bias"]:
                    nc.vector.tensor_scalar_add(
                        out=dst, in0=src, scalar1=qb4[32 * h:32 * h + 32, :]
                    )
                else:
                    nc.vector.tensor_copy(out=dst, in_=src)
            q2_ps = psA.tile([64, G, T], F32, tag="a", name="q2_ps")
            nc.tensor.matmul(
                q2_ps[:].rearrange("p g t -> p (g t)"), wq2[:], hhflat,
                start=True, stop=True,
            )
            for h in range(2):
                src = q2_ps[32 * h:32 * h + 32, :, :]
                dst = qblk2[32 * h:32 * h + 32, :, h, :]
                if flags["qkv_bias"]:
                    nc.vector.tensor_scalar_add(
                        out=dst, in0=src, scalar1=qb2[32 * h:32 * h + 32, :]
                    )
                else:
                    nc.vector.tensor_copy(out=dst, in_=src)
            k4_ps = psA.tile([128, G, T], F32, tag="a", name="k4_ps")
            nc.tensor.matmul(
                k4_ps[:].rearrange("p g t -> p (g t)"), wk4[:], hhflat,
                start=True, stop=True,
            )
            if flags["qkv_bias"]:
                nc.vector.tensor_scalar_add(out=k4sb[:], in0=k4_ps[:], scalar1=kb4[:])
            else:
                nc.vector.tensor_copy(out=k4sb[:], in_=k4_ps[:])
            k2_ps = psA.tile([64, G, T], F32, tag="a", name="k2_ps")
            nc.tensor.matmul(
                k2_ps[:].rearrange("p g t -> p (g t)"), wk2[:], hhflat,
                start=True, stop=True,
            )
            if flags["qkv_bias"]:
                nc.vector.tensor_scalar_add(out=k2sb[:], in0=k2_ps[:], scalar1=kb2[:])
            else:
                nc.vector.tensor_copy(out=k2sb[:], in_=k2_ps[:])

            v_ps = psA.tile([T, G, DC], F32, tag="a", name="v_ps")
            for b in range(G):
                nc.tensor.matmul(
                    v_ps[:, b, :], hhT[:, b, :], wv[:],
                    start=True, stop=True, skip_group_check=True,
                )
            vv = v_ps[:].rearrange("t g (h e) -> t g h e", h=NH)
            if flags["qkv_bias"]:
                vb3 = vb_r[:].rearrange("o (h e) -> o h e", h=NH)
                vb4 = bass.AP(
                    tensor=vb3.tensor, offset=vb3.offset,
                    ap=[[0, T], [0, G], vb3.ap[1], vb3.ap[2]],
                )
                nc.vector.tensor_tensor(
                    out=v_sb[:, :, :, 0:DH], in0=vv, in1=vb4, op=ALU.add
                )
            else:
                nc.vector.tensor_copy(out=v_sb[:, :, :, 0:DH], in_=vv)

            # attention pairs: scores -> exp -> mask -> AV -> softmax divide
            oT_ps = psM.tile([DC, G, T], BF, tag="m", name="oT_ps")
            for p in range(2):
                sp = psS.tile([T, 1536], F32, tag="s", name="sp")
                for j in range(2):
                    b = 2 * p + j
                    off4 = 0 if j == 0 else 1024
                    off2 = 512 if j == 0 else 768
                    nc.tensor.matmul(
                        sp[:, off4:off4 + 512],
                        k4sb[:, b, :],
                        qblk4[:, b, :, :].rearrange("p h t -> p (h t)"),
                        start=True, stop=True, skip_group_check=True,
                    )
                    nc.tensor.matmul(
                        sp[:, off2:off2 + 256],
                        k2sb[:, b, :],
                        qblk2[:, b, :, :].rearrange("p h t -> p (h t)"),
                        start=True, stop=True, skip_group_check=True,
                    )
                eT = sbg.tile([T, 1536], BF, tag="eT", name="eT")
                nc.scalar.activation(out=eT[:], in_=sp[:], func=AF.Exp)
                mb = bass.AP(
                    tensor=mask01.tensor, offset=mask01.offset,
                    ap=[mask01.ap[0], [0, 12], mask01.ap[1]],
                )
                eTv = eT[:].rearrange("t (m c) -> t m c", c=T)
                nc.vector.tensor_tensor(out=eTv, in0=eTv, in1=mb, op=ALU.mult)

                o_ps = psO.tile([T, 2, NH, DH + 1], F32, tag="o", name="o_ps")
                for j in range(2):
                    b = 2 * p + j
                    for h in range(NH):
                        c = ecol(j, h)
                        nc.tensor.matmul(
                            o_ps[:, j, h, :], eT[:, c:c + T], v_sb[:, b, h, :],
                            start=True, stop=True, skip_group_check=True,
                        )
                recip = sbg.tile([T, 2, NH, 1], F32, tag="recip", name="recip")
                nc.vector.reciprocal(out=recip[:], in_=o_ps[:, :, :, DH:DH + 1])
                o_sb = sbg.tile([T, 2, NH, DH], BF, tag="o_sb", name="o_sb")
                nc.vector.tensor_tensor(
                    out=o_sb[:], in0=o_ps[:, :, :, 0:DH],
                    in1=recip[:].to_broadcast([T, 2, NH, DH]), op=ALU.mult,
                )
                for j in range(2):
                    b = 2 * p + j
                    nc.tensor.transpose(
                        out=oT_ps[:, b, :],
                        in_=o_sb[:, j, :, :].rearrange("t h e -> t (h e)"),
                        identity=ident[:],
                    )
            oT_sb = sbg.tile([DC, G, T], BF, tag="oT_sb", name="oT_sb")
            nc.vector.tensor_copy(out=oT_sb[:], in_=oT_ps[:])

            # attT = xT + Wo^T @ oT   (x rides in as an identity preload)
            att = psM.tile([D, G * T], F32, tag="m", name="att")
            nc.tensor.matmul(
                att[:], ident[:],
                xTb[:, G * g:G * (g + 1), :].rearrange("d g t -> d (g t)"),
                start=True, stop=False, skip_group_check=True,
            )
            nc.tensor.matmul(
                att[:], wo[:], oT_sb[:].rearrange("c g t -> c (g t)"),
                start=False, stop=True, skip_group_check=True,
            )
            x1o = xx1b[:, G * g:G * (g + 1), :].rearrange("d g t -> d (g t)")
            if flags["bo"]:
                nc.vector.tensor_scalar_add(out=x1o, in0=att[:], scalar1=bo_c[:])
            else:
                nc.vector.tensor_copy(out=x1o, in_=att[:])

        # ---- P2: LN2 stats ----------------------------------------------
        ln_stats(xx1b, stats2_dram)

        # ---- P3: feed-forward per group ----------------------------------
        for g in range(NG):
            h2T = sbg.tile([D, G, T], BF, tag="h2T", name="h2T")
            normalize(xx1b, stats2_dram, g, h2T, False)
            h2flat = h2T[:].rearrange("d g t -> d (g t)")
            r_sb = sbg.tile([128, 4, 512], BF, tag="r_sb", name="r_sb")
            for i in range(2):
                fp = psF.tile([128, 2, 512], F32, tag="f", name="fp")
                for c in range(2):
                    nc.tensor.matmul(
                        fp[:, c, :], w1[:, 128 * (2 * i + c):128 * (2 * i + c + 1)],
                        h2flat, start=True, stop=True, skip_group_check=True,
                    )
                if flags["b1"]:
                    for c in range(2):
                        nc.scalar.activation(
                            out=r_sb[:, 2 * i + c, :], in_=fp[:, c, :], func=AF.Relu,
                            bias=b1e[:, 2 * i + c:2 * i + c + 1],
                        )
                else:
                    nc.scalar.activation(
                        out=r_sb[:, 2 * i:2 * i + 2, :].rearrange("p c t -> p (c t)"),
                        in_=fp[:].rearrange("p c t -> p (c t)"), func=AF.Relu,
                    )
            fo = psM.tile([D, G * T], F32, tag="m", name="fo")
            nc.tensor.matmul(
                fo[:], ident[:],
                xx1b[:, G * g:G * (g + 1), :].rearrange("d g t -> d (g t)"),
                start=True, stop=False, skip_group_check=True,
            )
            for c in range(4):
                nc.tensor.matmul(
                    fo[:], w2c[:, c, :], r_sb[:, c, :],
                    start=False, stop=(c == 3), skip_group_check=True,
                )
            oo = outT[:, G * g:G * (g + 1), :].rearrange("d g t -> d (g t)")
            if flags["b2"]:
                nc.vector.tensor_scalar_add(out=oo, in0=fo[:], scalar1=b2_c[:])
            else:
                nc.vector.tensor_copy(out=oo, in_=fo[:])

        # ---- P4: transpose back + store ----------------------------------
        for b in range(bpc):
            nc.sync.dma_start_transpose(out=oS[:, b, :], in_=outT[:, b, :])
        nc.gpsimd.dma_start(out=y_ap.rearrange("b t d -> t b d"), in_=oS[:])

    repeat = int(os.environ.get("K_REPEAT", "1"))
    for _ in range(repeat):
        emit_once()


def build_program(weights, flags, bpc=BPC):
    nc = bacc.Bacc("TRN2", target_bir_lowering=False, debug=False)
    aps = {}
    aps["x"] = nc.dram_tensor("x", [bpc, T, D], F32, kind="ExternalInput").ap()
    aps["y"] = nc.dram_tensor("y", [bpc, T, D], F32, kind="ExternalOutput").ap()
    for name, arr in weights.items():
        dt = F32 if arr.dtype == np.float32 else BF
        aps[name] = nc.dram_tensor(name, list(arr.shape), dt, kind="ExternalInput").ap()
    with tile.TileContext(nc) as tc:
        with ExitStack() as ctx:
            _emit(ctx, tc, aps, flags, bpc)
    nc.compile()
    return nc


_CACHE = {}


def _get_program_and_maps(x, args):
    x = np.asarray(x, np.float32)
    weights, flags = _prep_weights(*args)
    key = tuple(sorted(flags.items()))
    if key not in _CACHE:
        _CACHE[key] = build_program(weights, flags)
    nc = _CACHE[key]
    in_maps = []
    for c in range(NCORES):
        m = {"x": np.ascontiguousarray(x[c * BPC:(c + 1) * BPC])}
        m.update(weights)
        in_maps.append(m)
    return nc, in_maps


def kernel(x, Wq, Wk, Wv, Wo, bo, W1, b1, W2, b2, g1, beta1, g2, beta2):
    nc, in_maps = _get_program_and_maps(
        x, (Wq, Wk, Wv, Wo, bo, W1, b1, W2, b2, g1, beta1, g2, beta2)
    )
    res = run_bass_kernel_spmd(nc, in_maps, list(range(NCORES)))
    out = np.concatenate([res.results[c]["y"] for c in range(NCORES)], axis=0)
    return out.astype(np.float32)


def run_traced(inputs):
    """Profiled run; returns BassKernelResults with exec_time_ns if available."""
    args = tuple(
        inputs[k]
        for k in ("Wq", "Wk", "Wv", "Wo", "bo", "W1", "b1", "W2", "b2",
                  "g1", "beta1", "g2", "beta2")
    )
    nc, in_maps = _get_program_and_maps(inputs["x"], args)
    return run_bass_kernel_spmd(nc, in_maps, list(range(NCORES)), trace=True)


# revision 25
# speedup vs baseline: 1.5293x; 1.1484x over previous
"""Trainium2 Bass/Tile kernel for a pre-norm causal decoder block.

Math (matches the jax reference):
    h   = LN1(x) * g1 + beta1
    q,k,v = per-head projections of h (D_HEAD=21, 6 heads)
    sT  = (k @ q^T) / sqrt(21)                       (scores, transposed)
    e   = exp(sT) * tril01                           (multiplicative causal mask)
    o   = (e^T @ [v | 1]) -> softmax denominator in the appended column
    x1  = x + (o / denom) @ Wo + bo
    out = x1 + relu(LN2(x1) @ (g2*W1) + b1eff) @ W2 + b2

Sharding: pure data parallelism, batch 512 -> 64 per core across 8 cores.

Layout strategy (per core) - the residual stream lives TRANSPOSED as
xT[d, (b, t)] so model-dim contractions never need PE transposes:
  - x is loaded with a casting transposing DMA (f32 -> bf16, "b t d -> d b t")
  - LN statistics are ones-vector matmuls over the partition (d) axis;
    mean/var/rstd are computed once for all 64 batches on [1, 8192] rows
    (rstd = exp(-0.5*ln(var+eps)) keeps the Act engine on a single
    activation-table set: natural_log_exp covers Ln/Exp/Relu)
  - scores use K-stacked stationaries (4 heads x 32 rows = 128) with a
    block-diagonal zero-padded moving operand, so one matmul emits four
    heads' scores; heads 4-5 go in a second K=64 matmul
  - the causal mask is a multiplicative 0/1 tril applied post-exp on DVE
    (no mask matmuls, no -inf arithmetic)
  - attention output o[t,(h,e+1)] carries the softmax denominator in an
    appended ones column of v; Wo and the FF matmuls are group-batched
    (moving N=512) and both residual adds ride as identity-matmul
    preloads of the PSUM accumulators
All matmul operands bf16, PSUM accumulation fp32.
"""

import os
import numpy as np
import ml_dtypes

from contextlib import ExitStack

import concourse.bass as bass
import concourse.bacc as bacc
import concourse.tile as tile
from concourse import mybir
from concourse.bass_utils import run_bass_kernel_spmd

BF = mybir.dt.bfloat16
F32 = mybir.dt.float32
NPBF = ml_dtypes.bfloat16

B, T, D = 512, 128, 128
NH, DH = 6, 21
DC = NH * DH  # 126
DFF = 512
NCORES = 8
BPC = B // NCORES  # 64 batches per core
G = 4              # batches per group
NG = BPC // G      # 16 groups
EPS = 1e-5
SM_SCALE = 1.0 / np.sqrt(np.float32(DH))

AF = mybir.ActivationFunctionType
ALU = mybir.AluOpType


def _bf(a):
    return np.ascontiguousarray(np.asarray(a, dtype=np.float32)).astype(NPBF)


def _prep_weights(Wq, Wk, Wv, Wo, bo, W1, b1, W2, b2, g1, beta1, g2, beta2):
    """Host-side folding/packing. Returns dict of named arrays + flags."""
    Wq = np.asarray(Wq, np.float64)
    Wk = np.asarray(Wk, np.float64)
    Wv = np.asarray(Wv, np.float64)
    g1 = np.asarray(g1, np.float64)
    g2 = np.asarray(g2, np.float64)
    beta1 = np.asarray(beta1, np.float64)
    beta2 = np.asarray(beta2, np.float64)
    W1 = np.asarray(W1, np.float64)

    Wq_f = g1[None, :, None] * Wq * SM_SCALE   # [h, d, e]
    Wk_f = g1[None, :, None] * Wk
    Wv_f = g1[None, :, None] * Wv

    # K-stacked projection weights: 4 heads (rows 32h..32h+20) and 2 heads
    wq4 = np.zeros((D, 128), np.float64)
    wk4 = np.zeros((D, 128), np.float64)
    wq2 = np.zeros((D, 64), np.float64)
    wk2 = np.zeros((D, 64), np.float64)
    for h in range(4):
        wq4[:, 32 * h:32 * h + DH] = Wq_f[h]
        wk4[:, 32 * h:32 * h + DH] = Wk_f[h]
    for h in range(2):
        wq2[:, 32 * h:32 * h + DH] = Wq_f[4 + h]
        wk2[:, 32 * h:32 * h + DH] = Wk_f[4 + h]
    wv = np.concatenate([Wv_f[h] for h in range(NH)], axis=1)  # [128, 126]

    # beta1 contributions (per-stack-row biases for q/k; per-(h,e) row for v)
    qb = np.einsum("d,hde->he", beta1, Wq) * SM_SCALE   # [6, 21]
    kb = np.einsum("d,hde->he", beta1, Wk)
    vb = np.einsum("d,hde->he", beta1, Wv)
    qb4 = np.zeros((128, 1), np.float64)
    kb4 = np.zeros((128, 1), np.float64)
    qb2 = np.zeros((64, 1), np.float64)
    kb2 = np.zeros((64, 1), np.float64)
    for h in range(4):
        qb4[32 * h:32 * h + DH, 0] = qb[h]
        kb4[32 * h:32 * h + DH, 0] = kb[h]
    for h in range(2):
        qb2[32 * h:32 * h + DH, 0] = qb[4 + h]
        kb2[32 * h:32 * h + DH, 0] = kb[4 + h]

    w1 = g2[:, None] * W1                     # [128, 512]
    b1_eff = np.asarray(b1, np.float64) + beta2 @ W1   # [512]
    w2c = np.asarray(W2, np.float64).reshape(4, 128, D).transpose(1, 0, 2)  # [128,4,128]

    mask01 = np.where(
        np.arange(T)[:, None] <= np.arange(T)[None, :], 1.0, 0.0
    )  # [s, t] keep where s <= t

    out = {
        "wq4": _bf(wq4), "wq2": _bf(wq2), "wk4": _bf(wk4), "wk2": _bf(wk2),
        "wv": _bf(wv), "wo": _bf(Wo), "w1": _bf(w1), "w2c": _bf(w2c),
        "mask01": _bf(mask01), "ident": _bf(np.eye(128)),
        "ones_bf": _bf(np.ones((D, 1))),
        "qb4": np.asarray(qb4, np.float32), "qb2": np.asarray(qb2, np.float32),
        "kb4": np.asarray(kb4, np.float32), "kb2": np.asarray(kb2, np.float32),
        "vb_r": np.asarray(vb.reshape(1, DC), np.float32),
        "bo_c": np.asarray(bo, np.float32).reshape(D, 1),
        "b2_c": np.asarray(b2, np.float32).reshape(D, 1),
        "beta1_c": np.asarray(beta1, np.float32).reshape(D, 1),
        "b1e": np.ascontiguousarray(
            np.asarray(b1_eff, np.float64).reshape(4, 128).T, dtype=np.float32
        ),  # [128, 4] per-partition relu bias per chunk
    }
    flags = {
        "qkv_bias": bool(np.any(beta1 != 0.0)),
        "beta1": bool(np.any(beta1 != 0.0)),
        "bo": bool(np.any(np.asarray(bo) != 0.0)),
        "b2": bool(np.any(np.asarray(b2) != 0.0)),
        "b1": bool(np.any(out["b1e"] != 0.0)),
    }
    return out, flags


def _emit(ctx, tc, aps, flags, bpc):
    nc = tc.nc
    x_ap = aps["x"]
    y_ap = aps["y"]

    singles = ctx.enter_context(tc.tile_pool(name="singles", bufs=1))
    sbg = ctx.enter_context(tc.tile_pool(name="sbg", bufs=2))
    psA = ctx.enter_context(tc.tile_pool(name="psA", bufs=1, space="PSUM"))
    psS = ctx.enter_context(tc.tile_pool(name="psS", bufs=1, space="PSUM"))
    psO = ctx.enter_context(tc.tile_pool(name="psO", bufs=1, space="PSUM"))
    psM = ctx.enter_context(tc.tile_pool(name="psM", bufs=1, space="PSUM"))
    psF = ctx.enter_context(tc.tile_pool(name="psF", bufs=1, space="PSUM"))

    def load_const(name, shape, dtype=BF):
        t = singles.tile(list(shape), dtype, tag=name, name=name)
        nc.sync.dma_start(out=t[:], in_=aps[name])
        return t

    wq4 = load_const("wq4", [D, 128])
    wq2 = load_const("wq2", [D, 64])
    wk4 = load_const("wk4", [D, 128])
    wk2 = load_const("wk2", [D, 64])
    wv = load_const("wv", [D, DC])
    wo = load_const("wo", [DC, D])
    w1 = load_const("w1", [D, DFF])
    w2c = load_const("w2c", [D, 4, D])
    mask01 = load_const("mask01", [T, T])
    ident = load_const("ident", [128, 128])
    ones_bf = load_const("ones_bf", [D, 1])
    if flags["qkv_bias"]:
        qb4 = load_const("qb4", [128, 1], F32)
        qb2 = load_const("qb2", [64, 1], F32)
        kb4 = load_const("kb4", [128, 1], F32)
        kb2 = load_const("kb2", [64, 1], F32)
        vb_r = load_const("vb_r", [1, DC], F32)
    if flags["beta1"]:
        beta1_c = load_const("beta1_c", [D, 1], F32)
    if flags["bo"]:
        bo_c = load_const("bo_c", [D, 1], F32)
    if flags["b2"]:
        b2_c = load_const("b2_c", [D, 1], F32)
    if flags["b1"]:
        b1e = load_const("b1e", [128, 4], F32)

    # ---- whole-core resident tensors -------------------------------------
    stage = singles.tile([T, bpc, D], BF, tag="stage")   # load/store staging
    xb, oS = stage, stage
    xTb = singles.tile([D, bpc, T], BF, tag="xTb")       # bf16 residual in
    xx1b = singles.tile([D, bpc, T], BF, tag="xx1b")     # post-attn residual
    outT = singles.tile([D, bpc, T], BF, tag="outT")     # final output (T)
    # LN stat rows (all on partition 0; shared by LN1/LN2 - the per-group
    # broadcasts read the DRAM copies)
    st_sums = singles.tile([1, NG * 512], BF, tag="st_sums")
    st_sq = singles.tile([1, NG * 512], BF, tag="st_sq")
    st_mu = singles.tile([1, NG * 512], BF, tag="st_mu")
    st_rstd = singles.tile([1, NG * 512], BF, tag="st_rstd")

    # block-diagonal moving operands for the K-stacked score matmuls;
    # off-block zeros are written once and never touched again
    qblk4 = singles.tile([128, G, 4, T], BF, tag="qblk4")
    qblk2 = singles.tile([64, G, 2, T], BF, tag="qblk2")
    v_sb = singles.tile([T, G, NH, DH + 1], BF, tag="v_sb")
    k4sb = singles.tile([128, G, T], BF, tag="k4sb")
    k2sb = singles.tile([64, G, T], BF, tag="k2sb")

    nc.vector.memset(qblk4[:], 0.0)
    nc.vector.memset(qblk2[:], 0.0)
    nc.gpsimd.memset(v_sb[:, :, :, DH:DH + 1], 1.0)
    eps_t = singles.tile([4, 1], F32, tag="eps")
    nc.vector.memset(eps_t[:], EPS)

    NQ = 4           # DMA quarters
    BQ = bpc // NQ   # 16 batches per quarter

    # eT column offset of head h for pair-slot j (see spair bank layout)
    def ecol(j, h):
        if j == 0:
            return 128 * h if h < 4 else 512 + 128 * (h - 4)
        return 1024 + 128 * h if h < 4 else 768 + 128 * (h - 4)

    stats1_dram = nc.dram_tensor("stats1_dram", [2, NG * 512], BF, kind="Internal").ap()
    stats2_dram = nc.dram_tensor("stats2_dram", [2, NG * 512], BF, kind="Internal").ap()

    def ln_stats(src, stats_dram):
        """Per-(b,t)-column mean/rstd over the partition (d) axis."""
        xsqg = None
        for g in range(NG):
            cols = slice(512 * g, 512 * (g + 1))
            if g % 2 == 0:
                xsqg = sbg.tile([D, 2 * G, T], BF, tag="xsqg", name="xsqg")
                s2 = src[:, G * g:G * (g + 2), :]
                nc.vector.tensor_tensor(out=xsqg[:], in0=s2, in1=s2, op=ALU.mult)
            s = src[:, G * g:G * (g + 1), :]
            stp = psM.tile([1, 512], F32, tag="m", name="stp")
            nc.tensor.matmul(
                stp[:], ones_bf[:],
                s.rearrange("d g t -> d (g t)"),
                start=True, stop=True, skip_group_check=True,
            )
            stp2 = psO.tile([1, 512], F32, tag="o", name="stp2")
            nc.tensor.matmul(
                stp2[:], ones_bf[:],
                xsqg[:, G * (g % 2):G * (g % 2 + 1), :].rearrange("d g t -> d (g t)"),
                start=True, stop=True, skip_group_check=True,
            )
            nc.vector.tensor_copy(out=st_sums[0:1, cols], in_=stp[:])
            nc.vector.tensor_copy(out=st_sq[0:1, cols], in_=stp2[:])
        nc.vector.tensor_scalar_mul(out=st_mu[:], in0=st_sums[:], scalar1=1.0 / D)
        nc.vector.tensor_scalar_mul(out=st_sq[:], in0=st_sq[:], scalar1=1.0 / D)
        nc.vector.tensor_mul(out=st_sums[:], in0=st_mu[:], in1=st_mu[:])
        nc.vector.tensor_sub(out=st_sq[:], in0=st_sq[:], in1=st_sums[:])
        nc.scalar.activation(out=st_sq[:], in_=st_sq[:], func=AF.Ln, bias=eps_t[0:1, :])
        nc.scalar.activation(out=st_rstd[:], in_=st_sq[:], func=AF.Exp, scale=-0.5)
        nc.sync.dma_start(out=stats_dram[0:1, :], in_=st_mu[:])
        nc.sync.dma_start(out=stats_dram[1:2, :], in_=st_rstd[:])

    def normalize(src, stats_dram, gp, out_t, add_beta1):
        """out_t[d, (2g t)] = (src - mu) * rstd  (+ beta1), for group pair gp."""
        cols = slice(1024 * gp, 1024 * (gp + 1))
        # DVE cannot broadcast across partitions: replicate the stat rows
        # (via DRAM, whose APs allow a zero partition step)
        rep = sbg.tile([D, 2, 1024], BF, tag="rep", name="rep")
        nc.sync.dma_start(
            out=rep[:, 0, :], in_=stats_dram[0:1, cols].to_broadcast([D, 1024])
        )
        nc.sync.dma_start(
            out=rep[:, 1, :], in_=stats_dram[1:2, cols].to_broadcast([D, 1024])
        )
        s = src[:, 2 * G * gp:2 * G * (gp + 1), :].rearrange("d g t -> d (g t)")
        o = out_t[:].rearrange("d g t -> d (g t)")
        nc.vector.tensor_tensor(out=o, in0=s, in1=rep[:, 0, :], op=ALU.subtract)
        nc.vector.tensor_tensor(out=o, in0=o, in1=rep[:, 1, :], op=ALU.mult)
        if add_beta1:
            nc.vector.tensor_scalar_add(out=o, in0=o, scalar1=beta1_c[:])

    def emit_once():
        # ---- P0: load + LN1 stats ----------------------------------------
        nc.gpsimd.dma_start(out=xb[:], in_=x_ap.rearrange("b t d -> t b d"))
        for b in range(bpc):
            nc.sync.dma_start_transpose(out=xTb[:, b, :], in_=xb[:, b, :])
        ln_stats(xTb, stats1_dram)

        # ---- P1: attention per group -------------------------------------
        hh2 = None
        for g in range(NG):
            if g % 2 == 0:
                hh2 = sbg.tile([D, 2 * G, T], BF, tag="hhT", name="hh2")
                normalize(xTb, stats1_dram, g // 2, hh2, flags["beta1"])
            hhT = hh2[:, G * (g % 2):G * (g % 2 + 1), :]
            hhflat = hhT.rearrange("d g t -> d (g t)")

            # q/k projections (K-stacked rows) + block-diag staging
            q4_ps = psA.tile([128, G, T], F32, tag="a", name="q4_ps")
            nc.tensor.matmul(
                q4_ps[:].rearrange("p g t -> p (g t)"), wq4[:], hhflat,
                start=True, stop=True,
            )
            for h in range(4):
                src = q4_ps[32 * h:32 * h + 32, :, :]
                dst = qblk4[32 * h:32 * h + 32, :, h, :]
                if flags["qkv_bias"]:
                    nc.vector.tensor_scalar_add(
                        out=dst, in0=src, scalar1=qb4[32 * h:32 * h + 32, :]
                    )
                else:
                    nc.vector.tensor_copy(out=dst, in_=src)
            q2_ps = psA.tile([64, G, T], F32, tag="a", name="q2_ps")
            nc.tensor.matmul(
                q2_ps[:].rearrange("p g t -> p (g t)"), wq2[:], hhflat,
                start=True, stop=True,
            )
            for h in range(2):
                src = q2_ps[32 * h:32 * h + 32, :, :]
                dst = qblk2[32 * h:32 * h + 32, :, h, :]
                if flags["qkv_bias"]:
                    nc.vector.tensor_scalar_add(
                        out=dst, in0=src, scalar1=qb2[32 * h:32 * h + 32, :]
                    )
                else:
                    nc.vector.tensor_copy(out=dst, in_=src)
            k4_ps = psA.tile([128, G, T], F32, tag="a", name="k4_ps")
            nc.tensor.matmul(
                k4_ps[:].rearrange("p g t -> p (g t)"), wk4[:], hhflat,
                start=True, stop=True,
            )
            if flags["qkv_bias"]:
                nc.vector.tensor_scalar_add(out=k4sb[:], in0=k4_ps[:], scalar1=kb4[:])
            else:
                nc.vector.tensor_copy(out=k4sb[:], in_=k4_ps[:])
            k2_ps = psA.tile([64, G, T], F32, tag="a", name="k2_ps")
            nc.tensor.matmul(
                k2_ps[:].rearrange("p g t -> p (g t)"), wk2[:], hhflat,
                start=True, stop=True,
            )
            if flags["qkv_bias"]:
                nc.vector.tensor_scalar_add(out=k2sb[:], in0=k2_ps[:], scalar1=kb2[:])
            else:
                nc.vector.tensor_copy(out=k2sb[:], in_=k2_ps[:])

            v_ps = psA.tile([T, G, DC], F32, tag="a", name="v_ps")
            for b in range(G):
                nc.tensor.matmul(
                    v_ps[:, b, :], hhT[:, b, :], wv[:],
                    start=True, stop=True, skip_group_check=True,
                )
            vv = v_ps[:].rearrange("t g (h e) -> t g h e", h=NH)
            if flags["qkv_bias"]:
                vb3 = vb_r[:].rearrange("o (h e) -> o h e", h=NH)
                vb4 = bass.AP(
                    tensor=vb3.tensor, offset=vb3.offset,
                    ap=[[0, T], [0, G], vb3.ap[1], vb3.ap[2]],
                )
                nc.vector.tensor_tensor(
                    out=v_sb[:, :, :, 0:DH], in0=vv, in1=vb4, op=ALU.add
                )
            else:
                nc.vector.tensor_copy(out=v_sb[:, :, :, 0:DH], in_=vv)

            # attention pairs: scores -> exp -> mask -> AV -> softmax divide
            oT_ps = psM.tile([DC, G, T], BF, tag="m", name="oT_ps")
            for p in range(2):
                sp = psS.tile([T, 1536], F32, tag="s", name="sp")
                for j in range(2):
                    b = 2 * p + j
                    off4 = 0 if j == 0 else 1024
                    off2 = 512 if j == 0 else 768
                    nc.tensor.matmul(
                        sp[:, off4:off4 + 512],
                        k4sb[:, b, :],
                        qblk4[:, b, :, :].rearrange("p h t -> p (h t)"),
                        start=True, stop=True, skip_group_check=True,
                    )
                    nc.tensor.matmul(
                        sp[:, off2:off2 + 256],
                        k2sb[:, b, :],
                        qblk2[:, b, :, :].rearrange("p h t -> p (h t)"),
                        start=True, stop=True, skip_group_check=True,
                    )
                eT = sbg.tile([T, 1536], BF, tag="eT", name="eT")
                nc.scalar.activation(out=eT[:], in_=sp[:], func=AF.Exp)
                mb = bass.AP(
                    tensor=mask01.tensor, offset=mask01.offset,
                    ap=[mask01.ap[0], [0, 12], mask01.ap[1]],
                )
                eTv = eT[:].rearrange("t (m c) -> t m c", c=T)
                nc.vector.tensor_tensor(out=eTv, in0=eTv, in1=mb, op=ALU.mult)

                o_ps = psO.tile([T, 2, NH, DH + 1], F32, tag="o", name="o_ps")
                for j in range(2):
                    b = 2 * p + j
                    for h in range(NH):
                        c = ecol(j, h)
                        nc.tensor.matmul(
                            o_ps[:, j, h, :], eT[:, c:c + T], v_sb[:, b, h, :],
                            start=True, stop=True, skip_group_check=True,
                        )
                recip = sbg.tile([T, 2, NH, 1], F32, tag="recip", name="recip")
                nc.vector.reciprocal(out=recip[:], in_=o_ps[:, :, :, DH:DH + 1])
                o_sb = sbg.tile([T, 2, NH, DH], BF, tag="o_sb", name="o_sb")
                nc.vector.tensor_tensor(
                    out=o_sb[:], in0=o_ps[:, :, :, 0:DH],
                    in1=recip[:].to_broadcast([T, 2, NH, DH]), op=ALU.mult,
                )
                for j in range(2):
                    b = 2 * p + j
                    nc.tensor.transpose(
                        out=oT_ps[:, b, :],
                        in_=o_sb[:, j, :, :].rearrange("t h e -> t (h e)"),
                        identity=ident[:],
                    )
            oT_sb = sbg.tile([DC, G, T], BF, tag="oT_sb", name="oT_sb")
            nc.vector.tensor_copy(out=oT_sb[:], in_=oT_ps[:])

            # x1T = xT + Wo^T @ oT  (mixed f32-psum + bf16 residual add)
            att = psM.tile([D, G * T], F32, tag="m", name="att")
            nc.tensor.matmul(
                att[:], wo[:], oT_sb[:].rearrange("c g t -> c (g t)"),
                start=True, stop=True, skip_group_check=True,
            )
            x1o = xx1b[:, G * g:G * (g + 1), :].rearrange("d g t -> d (g t)")
            xres = xTb[:, G * g:G * (g + 1), :].rearrange("d g t -> d (g t)")
            nc.vector.tensor_tensor(out=x1o, in0=att[:], in1=xres, op=ALU.add)
            if flags["bo"]:
                nc.vector.tensor_scalar_add(out=x1o, in0=x1o, scalar1=bo_c[:])

        # ---- P2: LN2 stats ----------------------------------------------
        ln_stats(xx1b, stats2_dram)

        # ---- P3: feed-forward per group ----------------------------------
        h22 = None
        for g in range(NG):
            if g % 2 == 0:
                h22 = sbg.tile([D, 2 * G, T], BF, tag="h2T", name="h22")
                normalize(xx1b, stats2_dram, g // 2, h22, False)
            h2T = h22[:, G * (g % 2):G * (g % 2 + 1), :]
            h2flat = h2T.rearrange("d g t -> d (g t)")
            r_sb = sbg.tile([128, 4, 512], BF, tag="r_sb", name="r_sb")
            for i in range(2):
                fp = psF.tile([128, 2, 512], F32, tag="f", name="fp")
                for c in range(2):
                    nc.tensor.matmul(
                        fp[:, c, :], w1[:, 128 * (2 * i + c):128 * (2 * i + c + 1)],
                        h2flat, start=True, stop=True, skip_group_check=True,
                    )
                if flags["b1"]:
                    for c in range(2):
                        nc.scalar.activation(
                            out=r_sb[:, 2 * i + c, :], in_=fp[:, c, :], func=AF.Relu,
                            bias=b1e[:, 2 * i + c:2 * i + c + 1],
                        )
                else:
                    nc.scalar.activation(
                        out=r_sb[:, 2 * i:2 * i + 2, :].rearrange("p c t -> p (c t)"),
                        in_=fp[:].rearrange("p c t -> p (c t)"), func=AF.Relu,
                    )
            fo = psM.tile([D, G * T], F32, tag="m", name="fo")
            for c in range(4):
                nc.tensor.matmul(
                    fo[:], w2c[:, c, :], r_sb[:, c, :],
                    start=(c == 0), stop=(c == 3), skip_group_check=True,
                )
            oo = outT[:, G * g:G * (g + 1), :].rearrange("d g t -> d (g t)")
            x1res = xx1b[:, G * g:G * (g + 1), :].rearrange("d g t -> d (g t)")
            nc.vector.tensor_tensor(out=oo, in0=fo[:], in1=x1res, op=ALU.add)
            if flags["b2"]:
                nc.vector.tensor_scalar_add(out=oo, in0=oo, scalar1=b2_c[:])

        # ---- P4: transpose back + store ----------------------------------
        for b in range(bpc):
            nc.sync.dma_start_transpose(out=oS[:, b, :], in_=outT[:, b, :])
        nc.gpsimd.dma_start(out=y_ap.rearrange("b t d -> t b d"), in_=oS[:])

    repeat = int(os.environ.get("K_REPEAT", "1"))
    for _ in range(repeat):
        emit_once()


def build_program(weights, flags, bpc=BPC):
    nc = bacc.Bacc("TRN2", target_bir_lowering=False, debug=False)
    aps = {}
    aps["x"] = nc.dram_tensor("x", [bpc, T, D], F32, kind="ExternalInput").ap()
    aps["y"] = nc.dram_tensor("y", [bpc, T, D], F32, kind="ExternalOutput").ap()
    for name, arr in weights.items():
        dt = F32 if arr.dtype == np.float32 else BF
        aps[name] = nc.dram_tensor(name, list(arr.shape), dt, kind="ExternalInput").ap()
    with tile.TileContext(nc) as tc:
        with ExitStack() as ctx:
            _emit(ctx, tc, aps, flags, bpc)
    nc.compile()
    return nc


_CACHE = {}


def _get_program_and_maps(x, args):
    x = np.asarray(x, np.float32)
    weights, flags = _prep_weights(*args)
    key = tuple(sorted(flags.items()))
    if key not in _CACHE:
        _CACHE[key] = build_program(weights, flags)
    nc = _CACHE[key]
    in_maps = []
    for c in range(NCORES):
        m = {"x": np.ascontiguousarray(x[c * BPC:(c + 1) * BPC])}
        m.update(weights)
        in_maps.append(m)
    return nc, in_maps


def kernel(x, Wq, Wk, Wv, Wo, bo, W1, b1, W2, b2, g1, beta1, g2, beta2):
    nc, in_maps = _get_program_and_maps(
        x, (Wq, Wk, Wv, Wo, bo, W1, b1, W2, b2, g1, beta1, g2, beta2)
    )
    res = run_bass_kernel_spmd(nc, in_maps, list(range(NCORES)))
    out = np.concatenate([res.results[c]["y"] for c in range(NCORES)], axis=0)
    return out.astype(np.float32)


def run_traced(inputs):
    """Profiled run; returns BassKernelResults with exec_time_ns if available."""
    args = tuple(
        inputs[k]
        for k in ("Wq", "Wk", "Wv", "Wo", "bo", "W1", "b1", "W2", "b2",
                  "g1", "beta1", "g2", "beta2")
    )
    nc, in_maps = _get_program_and_maps(inputs["x"], args)
    return run_bass_kernel_spmd(nc, in_maps, list(range(NCORES)), trace=True)
